# revision 3
# baseline (speedup 1.0000x reference)
"""Trainium2 Bass kernel for ExportFriendlyRelPositionMultiHeadAttentionLongformer.

Sharding: 8 cores = batch(2) x sequence(4). Each core computes 2048 query
tokens (with a 128-token key/value halo) across all 8 heads, fully on-chip:
QKV projections -> banded rel-position attention -> output projection.

The relative-position band term bd[i,j] = (q_i+v)·p_j needs a per-row shift
("skew") to align with the QK^T band scores. Compute engines cannot do
per-partition shifts and SBUF-side diagonal DMA access patterns are
mis-lowered (descriptor bases are partition-aligned), so the skew goes
through a DRAM scratch: diagonal write (DRAM is flat-addressed, arbitrary
strides are legal) + rectangular read-back. Off-band scratch is pre-filled
with -60000 so the skew-add also applies the band mask; softmax needs no
max-subtraction (scores are O(1) by construction).
"""
import contextlib

import numpy as np
import ml_dtypes

import concourse.bass as bass
import concourse.bacc as bacc
import concourse.tile as tile
import concourse.mybir as mybir
from concourse.bass_utils import run_bass_kernel_spmd
from concourse import masks

B, T, D, H, DK, W = 2, 8192, 512, 8, 64, 128
NCORES = 8
TC = T // 4          # tokens per core: 2048
NCH = TC // W        # chunks per core: 16
HALO = TC + 2 * W    # 2304
BAND = 2 * W + 1     # 257
SKW = 3 * W          # skewed row width: 384
NEG = -60000.0       # band mask value (fits fp16; exp() underflows to 0)
HS = W * SKW + SKW   # DRAM scratch stride per head: 49536 = 128*387
NBUF = 2             # skew scratch double-buffer
KT = D // 128        # 4 k/m tiles per 512-dim
NV = HALO // 128     # 18 value tiles

f32 = mybir.dt.float32
f16 = mybir.dt.float16
AF = mybir.ActivationFunctionType
ALU = mybir.AluOpType

_CACHED = {}


def _build():
    nc = bacc.Bacc("TRN2", target_bir_lowering=False, num_devices=NCORES)

    # ---- per-core inputs (host pre-sliced/transposed, fp16 unless noted) ----
    qT_in = nc.declare_dram_parameter("qT_in", [D, TC], f16, isOutput=False)
    kT_in = nc.declare_dram_parameter("kT_in", [D, HALO], f16, isOutput=False)
    vT_in = nc.declare_dram_parameter("vT_in", [D, HALO], f16, isOutput=False)
    WqT = nc.declare_dram_parameter("WqT", [D, D], f16, isOutput=False)
    WkT = nc.declare_dram_parameter("WkT", [D, D], f16, isOutput=False)
    WvT = nc.declare_dram_parameter("WvT", [D, D], f16, isOutput=False)
    WoT = nc.declare_dram_parameter("WoT", [D, D], f16, isOutput=False)
    qu_b = nc.declare_dram_parameter("qu_b", [D, 1], f32, isOutput=False)
    qv_b = nc.declare_dram_parameter("qv_b", [D, 1], f32, isOutput=False)
    bk_b = nc.declare_dram_parameter("bk_b", [D, 1], f32, isOutput=False)
    bo_b = nc.declare_dram_parameter("bo_b", [D, 1], f32, isOutput=False)
    pT = nc.declare_dram_parameter("pT", [D, BAND], f16, isOutput=False)
    edgeL = nc.declare_dram_parameter("edgeL", [W, SKW], f16, isOutput=False)
    edgeR = nc.declare_dram_parameter("edgeR", [W, SKW], f16, isOutput=False)
    outT = nc.declare_dram_parameter("outT", [D, TC], f32, isOutput=True)

    scr = nc.dram_tensor("skew_scr", [NBUF * H * HS], f16)
    scr_ap = scr[:]

    with tile.TileContext(nc) as tc:
        ctx = contextlib.ExitStack()
        with ctx:
            wpool = ctx.enter_context(tc.tile_pool(name="weights", bufs=1))
            apool = ctx.enter_context(tc.tile_pool(name="arenas", bufs=1))
            cpool = ctx.enter_context(tc.tile_pool(name="consts", bufs=1))

            # ---------------- constants / weights ----------------
            w_q = [wpool.tile([128, D], f16, tag=f"wq{i}", name=f"wq{i}") for i in range(KT)]
            w_k = [wpool.tile([128, D], f16, tag=f"wk{i}", name=f"wk{i}") for i in range(KT)]
            w_v = [wpool.tile([128, D], f16, tag=f"wv{i}", name=f"wv{i}") for i in range(KT)]
            w_o = [wpool.tile([128, D], f16, tag=f"wo{i}", name=f"wo{i}") for i in range(KT)]
            for i in range(KT):
                nc.gpsimd.dma_start(w_q[i][:], WqT[bass.ts(i, 128), :])
                nc.gpsimd.dma_start(w_k[i][:], WkT[bass.ts(i, 128), :])
                nc.gpsimd.dma_start(w_v[i][:], WvT[bass.ts(i, 128), :])
                nc.gpsimd.dma_start(w_o[i][:], WoT[bass.ts(i, 128), :])
            qu_bias = [cpool.tile([128, 1], f32, tag=f"qub{i}", name=f"qub{i}") for i in range(KT)]
            qv_bias = [cpool.tile([128, 1], f32, tag=f"qvb{i}", name=f"qvb{i}") for i in range(KT)]
            bk_bias = [cpool.tile([128, 1], f32, tag=f"bkb{i}", name=f"bkb{i}") for i in range(KT)]
            bo_bias = [cpool.tile([128, 1], f32, tag=f"bob{i}", name=f"bob{i}") for i in range(KT)]
            for i in range(KT):
                nc.gpsimd.dma_start(qu_bias[i][:], qu_b[bass.ts(i, 128), :])
                nc.gpsimd.dma_start(qv_bias[i][:], qv_b[bass.ts(i, 128), :])
                nc.gpsimd.dma_start(bk_bias[i][:], bk_b[bass.ts(i, 128), :])
                nc.gpsimd.dma_start(bo_bias[i][:], bo_b[bass.ts(i, 128), :])
            p_sb = [cpool.tile([128, BAND], f16, tag=f"pt{i}", name=f"pt{i}") for i in range(KT)]
            for i in range(KT):
                nc.gpsimd.dma_start(p_sb[i][:], pT[bass.ts(i, 128), :])
            edgeL_sb = cpool.tile([W, SKW], f16, tag="egl")
            edgeR_sb = cpool.tile([W, SKW], f16, tag="egr")
            nc.gpsimd.dma_start(edgeL_sb[:], edgeL[:])
            nc.gpsimd.dma_start(edgeR_sb[:], edgeR[:])

            ident = cpool.tile([128, 128], f16, tag="ident")
            masks.make_identity(nc, ident[:])

            # fill skew scratch with NEG: each head-buf region is HS=128*387
            # elements, written as 128 rows of 387 from a NEG-filled tile.
            negt = cpool.tile([128, 387], f16, tag="negt")
            nc.vector.memset(negt[:], NEG)
            nap = negt[:]
            nsrc = bass.AP(tensor=nap.tensor, offset=nap.offset,
                           ap=[[387, 128], [1, 387]])
            for bu in range(NBUF):
                for h in range(H):
                    dstf = bass.AP(tensor=scr_ap.tensor, offset=(bu * H + h) * HS,
                                   ap=[[387, 128], [1, 387]])
                    nc.gpsimd.dma_start(dstf, nsrc)

            # ---------------- arenas ----------------
            quT = [apool.tile([128, TC], f16, tag=f"quT{i}", name=f"quT{i}") for i in range(KT)]
            qvT = [apool.tile([128, TC], f16, tag=f"qvT{i}", name=f"qvT{i}") for i in range(KT)]
            kT = [apool.tile([128, HALO], f16, tag=f"kT{i}", name=f"kT{i}") for i in range(KT)]
            v_sb = [apool.tile([128, D], f16, tag=f"v{i}", name=f"v{i}") for i in range(NV)]
            ctxT = [apool.tile([128, TC], f16, tag=f"ctxT{i}", name=f"ctxT{i}") for i in range(KT)]

            # ---------------- projections ----------------
            with tc.tile_pool(name="pin", bufs=1) as pin, \
                 tc.tile_pool(name="pps", bufs=2, space="PSUM") as pps:
                # Q projection -> quT/qvT arenas (transposed layout [dout, t])
                xq = [pin.tile([128, TC], f16, tag=f"xq{i}", name=f"xq{i}") for i in range(KT)]
                for i in range(KT):
                    nc.gpsimd.dma_start(xq[i][:], qT_in[bass.ts(i, 128), :])
                for m in range(KT):
                    for tb in range(TC // 512):
                        ps = pps.tile([128, 512], f32, tag="proj")
                        for kk in range(KT):
                            nc.tensor.matmul(ps[:], w_q[kk][:, bass.ts(m, 128)],
                                             xq[kk][:, bass.ts(tb, 512)],
                                             start=(kk == 0), stop=(kk == KT - 1))
                        nc.scalar.activation(quT[m][:, bass.ts(tb, 512)], ps[:],
                                             AF.Identity, bias=qu_bias[m][:])
                        nc.vector.tensor_scalar_add(qvT[m][:, bass.ts(tb, 512)],
                                                    ps[:], qv_bias[m][:])
                # K projection -> kT arena
                xk = [pin.tile([128, HALO], f16, tag=f"xk{i}", name=f"xk{i}") for i in range(KT)]
                for i in range(KT):
                    nc.gpsimd.dma_start(xk[i][:], kT_in[bass.ts(i, 128), :])
                for m in range(KT):
                    for tb in range(HALO // 384):
                        ps = pps.tile([128, 384], f32, tag="projk")
                        for kk in range(KT):
                            nc.tensor.matmul(ps[:], w_k[kk][:, bass.ts(m, 128)],
                                             xk[kk][:, bass.ts(tb, 384)],
                                             start=(kk == 0), stop=(kk == KT - 1))
                        nc.scalar.activation(kT[m][:, bass.ts(tb, 384)], ps[:],
                                             AF.Identity, bias=bk_bias[m][:])
                # V projection -> v tiles (natural layout [t, dout])
                xv = [pin.tile([128, HALO], f16, tag=f"xv{i}", name=f"xv{i}") for i in range(KT)]
                for i in range(KT):
                    nc.gpsimd.dma_start(xv[i][:], vT_in[bass.ts(i, 128), :])
                for mt in range(NV):
                    ps = pps.tile([128, 512], f32, tag="proj")
                    for kk in range(KT):
                        nc.tensor.matmul(ps[:], xv[kk][:, bass.ts(mt, 128)],
                                         w_v[kk][:],
                                         start=(kk == 0), stop=(kk == KT - 1))
                    nc.scalar.copy(v_sb[mt][:], ps[:])

            # ---------------- attention ----------------
            with tc.tile_pool(name="att", bufs=2) as att, \
                 tc.tile_pool(name="atr", bufs=2) as atr, \
                 tc.tile_pool(name="ps_s", bufs=3, space="PSUM") as ps_s, \
                 tc.tile_pool(name="ps_t", bufs=2, space="PSUM") as ps_t:
                for c in range(NCH):
                    bufi = c % NBUF
                    bd_sb = att.tile([128, H * BAND], f16, tag="bd_sb")
                    e_sb = att.tile([128, H * SKW], f16, tag="e_sb")

                    # phase A: bd = (q+v)·p for all heads, 2-head row-packed
                    for pr in range(H // 2):
                        t0, t1 = 2 * pr, 2 * pr + 1
                        bd0 = ps_s.tile([128, BAND], f32, tag="s")
                        bd1 = ps_s.tile([128, BAND], f32, tag="s")
                        nc.tensor.matmul(bd0[:], qvT[pr][0:64, bass.ts(c, 128)],
                                         p_sb[pr][0:64, :], start=True, stop=True)
                        nc.tensor.matmul(bd1[:], qvT[pr][64:128, bass.ts(c, 128)],
                                         p_sb[pr][64:128, :], start=True, stop=True)
                        nc.scalar.copy(bd_sb[:, bass.ds(t0 * BAND, BAND)], bd0[:])
                        nc.scalar.copy(bd_sb[:, bass.ds(t1 * BAND, BAND)], bd1[:])

                    # skew through DRAM scratch (diag write + rect read-back)
                    bap = bd_sb[:]
                    src_w = bass.AP(tensor=bap.tensor, offset=bap.offset,
                                    ap=[[H * BAND, 128], [BAND, H], [1, BAND]])
                    dst_w = bass.AP(tensor=scr_ap.tensor, offset=bufi * H * HS,
                                    ap=[[SKW + 1, 128], [HS, H], [1, BAND]])
                    nc.gpsimd.dma_start(dst_w, src_w)
                    eap = e_sb[:]
                    dst_r = bass.AP(tensor=eap.tensor, offset=eap.offset,
                                    ap=[[H * SKW, 128], [SKW, H], [1, SKW]])
                    src_r = bass.AP(tensor=scr_ap.tensor, offset=bufi * H * HS,
                                    ap=[[SKW, 128], [HS, H], [1, SKW]])
                    nc.sync.dma_start(dst_r, src_r)

                    # phase B: QK^T, softmax, PV per head pair
                    for pr in range(H // 2):
                        s0 = ps_s.tile([128, SKW], f32, tag="s")
                        s1 = ps_s.tile([128, SKW], f32, tag="s")
                        nc.tensor.matmul(s0[:], quT[pr][0:64, bass.ts(c, 128)],
                                         kT[pr][0:64, bass.ds(c * W, SKW)],
                                         start=True, stop=True)
                        nc.tensor.matmul(s1[:], quT[pr][64:128, bass.ts(c, 128)],
                                         kT[pr][64:128, bass.ds(c * W, SKW)],
                                         start=True, stop=True)
                        cps = ps_t.tile([128, 128], f32, tag="ctx")
                        for h, s_ps in ((2 * pr, s0), (2 * pr + 1, s1)):
                            nc.vector.tensor_tensor(
                                s_ps[:], s_ps[:], e_sb[:, bass.ds(h * SKW, SKW)],
                                op=ALU.add)
                            if c == 0:
                                nc.vector.tensor_tensor(s_ps[:], s_ps[:],
                                                        edgeL_sb[:], op=ALU.add)
                            if c == NCH - 1:
                                nc.vector.tensor_tensor(s_ps[:], s_ps[:],
                                                        edgeR_sb[:], op=ALU.add)
                            a_sb = atr.tile([128, SKW], f16, tag="a_sb")
                            dnm = atr.tile([128, 1], f32, tag="dnm")
                            nc.scalar.activation(a_sb[:], s_ps[:], AF.Exp,
                                                 accum_out=dnm[:])
                            rcp = atr.tile([128, 1], f32, tag="rcp")
                            nc.vector.reciprocal(rcp[:], dnm[:])
                            an_sb = atr.tile([128, SKW], f16, tag="an_sb")
                            nc.vector.tensor_scalar_mul(an_sb[:], a_sb[:], rcp[:])

                            po = 64 * (h % 2)
                            for m in range(3):
                                trp = ps_t.tile([128, 128], f16, tag="tr")
                                nc.tensor.transpose(
                                    trp[:], an_sb[:, bass.ts(m, 128)], ident[:])
                                trs = atr.tile([128, 128], f16, tag="trs")
                                if m == 1:
                                    nc.scalar.copy(trs[:], trp[:])
                                else:
                                    nc.vector.tensor_copy(trs[:], trp[:])
                                nc.tensor.matmul(
                                    cps[po:po + 64, :],
                                    v_sb[c + m][:, bass.ds(h * DK, DK)],
                                    trs[:], start=(m == 0), stop=(m == 2))
                        nc.scalar.copy(ctxT[pr][:, bass.ts(c, 128)], cps[:])

            # ---------------- output projection ----------------
            with tc.tile_pool(name="osbp", bufs=2) as osbp, \
                 tc.tile_pool(name="ops", bufs=2, space="PSUM") as ops:
                for m in range(KT):
                    for tb in range(TC // 512):
                        ps = ops.tile([128, 512], f32, tag="oproj")
                        for kk in range(KT):
                            nc.tensor.matmul(ps[:], w_o[kk][:, bass.ts(m, 128)],
                                             ctxT[kk][:, bass.ts(tb, 512)],
                                             start=(kk == 0), stop=(kk == KT - 1))
                        osb = osbp.tile([128, 512], f32, tag="osb")
                        nc.scalar.activation(osb[:], ps[:], AF.Identity,
                                             bias=bo_bias[m][:])
                        nc.gpsimd.dma_start(
                            outT[bass.ts(m, 128), bass.ts(tb, 512)], osb[:])
    nc.compile()
    return nc


def _prep_inputs(query, key_in, value, pad_mask, pos_emb, Wq, bq, Wk, bk, Wv, bv,
                 Wpos, pos_bias_u, pos_bias_v, Wout, bout):
    inv = 0.125  # 1/sqrt(DK)
    f16n = ml_dtypes.float16 if hasattr(ml_dtypes, "float16") else np.float16

    WqT = np.ascontiguousarray(Wq.T * inv).astype(f16n)   # [din, dout]
    WkT = np.ascontiguousarray(Wk.T).astype(f16n)
    WvT = np.ascontiguousarray(Wv.T).astype(f16n)
    WoT = np.ascontiguousarray(Wout.T).astype(f16n)
    qu_b = (bq * inv + pos_bias_u.reshape(D) * inv).reshape(D, 1).astype(np.float32)
    qv_b = (bq * inv + pos_bias_v.reshape(D) * inv).reshape(D, 1).astype(np.float32)
    bk_b = bk.reshape(D, 1).astype(np.float32)
    p = pos_emb.reshape(BAND, D) @ Wpos.T                 # [257, D]
    pT = np.ascontiguousarray(p.T).astype(f16n)           # [D, 257]
    bo_b = (bv @ Wout.T + bout).reshape(D, 1).astype(np.float32)

    cols = np.arange(SKW)[None, :]
    eL = np.broadcast_to(np.where(cols < W, np.float32(NEG), np.float32(0.0)),
                         (W, SKW))
    eR = np.broadcast_to(np.where(cols >= 2 * W, np.float32(NEG), np.float32(0.0)),
                         (W, SKW))
    zeros = np.zeros((W, SKW), np.float32)

    in_maps = []
    for core in range(NCORES):
        b, q = divmod(core, 4)
        t0 = q * TC
        qT = np.ascontiguousarray(query[b, t0:t0 + TC, :].T).astype(f16n)
        lo, hi = t0 - W, t0 + TC + W
        kpad = np.zeros((HALO, D), np.float32)
        vpad = np.zeros((HALO, D), np.float32)
        clo, chi = max(lo, 0), min(hi, T)
        kpad[clo - lo:chi - lo] = key_in[b, clo:chi]
        vpad[clo - lo:chi - lo] = value[b, clo:chi]
        in_maps.append({
            "qT_in": qT,
            "kT_in": np.ascontiguousarray(kpad.T).astype(f16n),
            "vT_in": np.ascontiguousarray(vpad.T).astype(f16n),
            "WqT": WqT, "WkT": WkT, "WvT": WvT, "WoT": WoT,
            "qu_b": qu_b, "qv_b": qv_b, "bk_b": bk_b, "bo_b": bo_b,
            "pT": pT,
            "edgeL": np.ascontiguousarray(eL if t0 == 0 else zeros).astype(f16n),
            "edgeR": np.ascontiguousarray(eR if t0 + TC == T else zeros).astype(f16n),
        })
    return in_maps


def _reference_numpy(query, key_in, value, pad_mask, pos_emb, Wq, bq, Wk, bk, Wv,
                     bv, Wpos, pos_bias_u, pos_bias_v, Wout, bout):
    """Slow but general host fallback (only used if pad_mask has any True)."""
    b, t, d = query.shape
    h, dk, w = H, DK, W
    q = (query @ Wq.T + bq).reshape(b, t, h, dk).transpose(0, 2, 1, 3)
    k = (key_in @ Wk.T + bk).reshape(b, t, h, dk).transpose(0, 2, 1, 3)
    v = (value @ Wv.T + bv).reshape(b, t, h, dk).transpose(0, 2, 1, 3)
    p = (pos_emb.reshape(2 * w + 1, d) @ Wpos.T).reshape(2 * w + 1, h, dk)
    p = p.transpose(1, 0, 2)
    kp = np.pad(k, ((0, 0), (0, 0), (w, w), (0, 0)))
    vp = np.pad(v, ((0, 0), (0, 0), (w, w), (0, 0)))
    key_bias = np.where(pad_mask, -10000.0, 0.0)
    key_bias = np.pad(key_bias, ((0, 0), (w, w)), constant_values=-1e9)
    out = np.zeros((b, t, d), np.float32)
    for bi in range(b):
        for hi in range(h):
            qu = q[bi, hi] + pos_bias_u[hi]
            qv = q[bi, hi] + pos_bias_v[hi]
            bd = qv @ p[hi].T
            ctx = np.zeros((t, dk), np.float32)
            for ti in range(t):
                ks = kp[bi, hi, ti:ti + 2 * w + 1]
                sc = (qu[ti] @ ks.T + bd[ti]) / 8.0 + key_bias[bi, ti:ti + 2 * w + 1]
                sc = sc - sc.max()
                e = np.exp(sc)
                a = e / e.sum()
                ctx[ti] = a @ vp[bi, hi, ti:ti + 2 * w + 1]
            if pad_mask[bi].any():
                ctx[pad_mask[bi]] = 0.0
            out[bi, :, hi * dk:(hi + 1) * dk] = ctx
    return (out @ Wout.T + bout).astype(np.float32)


def kernel(**inputs):
    inputs = {k: np.asarray(v) for k, v in inputs.items()}
    if inputs["pad_mask"].any():
        return _reference_numpy(**inputs)

    if "nc" not in _CACHED:
        _CACHED["nc"] = _build()
    nc = _CACHED["nc"]
    in_maps = _prep_inputs(**inputs)
    res = run_bass_kernel_spmd(nc, in_maps, list(range(NCORES)))
    out = np.zeros((B, T, D), np.float32)
    for core in range(NCORES):
        b, q = divmod(core, 4)
        t0 = q * TC
        out[b, t0:t0 + TC, :] = res.results[core]["outT"].T
    return out
